# revision 16
# baseline (speedup 1.0000x reference)
"""Trainium2 Bass kernel for the LIF (leaky integrate-and-fire) scan problem.

Reference semantics (bit-exact fp32, validated 0/26M spike diffs vs jax):
    v = 0
    for t in range(L):
        v1 = v + (v * (-0.05) + I[:, t])     # == v + (-v/20 + I_t) bit-exactly
        s[:, t] = (v1 >= 1.0)
        v = 0 if s[:, t] else v1
    hard_latency = argmax(s, axis=1)         # first spike index (0 if none)
    soft_latency = sum(s*t) / (sum(s) + 1e-6)

Sharding: pure data parallel, 8 cores x 8192 rows (no communication).

Per-core layout: rows sit 64 per partition, time contiguous:
X[p, j*400 + t] = I[64p + j, t]. The recurrence runs as 399 x 3 vector-engine
instructions on [128, 64] stride-400 slices (u_t overwrites I_t in place;
r = post-reset potential in a small ping tile):
    e   = (r * -0.05) + I_t          scalar_tensor_tensor
    u_t = r + e                      tensor_tensor      (in place over I_t)
    r   = (u_t < 1) * u_t            scalar_tensor_tensor
This reproduces the reference's fp32 rounding sequence exactly.

Stats are bulk ops on the finished u array:
    X <- (u >= 1) * (t - 400)        per row-block stt, in place
    min over t  -> first_spike - 400       (0 if none -> first = 400 -> hard 0)
    sum over t  -> sum(s*t) - 400*sum(s)
    X <- (X < -0.5)                  == spikes, in place (grouped for DMA-out)
    sum over t  -> sum(s)
Raw Bass (no Tile scheduler): this container's walrus enforces one sync-wait
slot per instruction, so all waits are standalone wait_ge instructions with
hand-placed semaphores. Input DMA is chunked along time and output DMA along
row-blocks so both overlap compute.
"""

import numpy as np

B, L = 65536, 400
NCORES = 8
BS = B // NCORES  # 8192 rows per core
P = 128
J = BS // P  # 64 rows per partition
TH = 1.0
NEG_INV_TAU = -0.05

IN_CHUNKS = 4
OUT_GROUPS = 4

_PROGRAM_CACHE = {}


def _build_program():
    import concourse.bass as bass
    import concourse.mybir as mybir

    f32 = mybir.dt.float32
    i32 = mybir.dt.int32
    Alu = mybir.AluOpType
    Ax = mybir.AxisListType

    nc = bass.Bass()

    I_in = nc.dram_tensor("I", [BS, L], f32, kind="ExternalInput")
    trampm_in = nc.dram_tensor("trampm", [P, L], f32, kind="ExternalInput")
    spikes_out = nc.dram_tensor("spikes", [BS, L], f32, kind="ExternalOutput")
    hard_out = nc.dram_tensor("hard", [BS], i32, kind="ExternalOutput")
    ssum_out = nc.dram_tensor("ssum", [BS], f32, kind="ExternalOutput")
    sts_out = nc.dram_tensor("sts", [BS], f32, kind="ExternalOutput")

    I_r = I_in.rearrange("(p j) t -> p j t", p=P)
    S_r = spikes_out.rearrange("(p j) t -> p j t", p=P)
    hard_r = hard_out.rearrange("(p j) -> p j", p=P)
    ssum_r = ssum_out.rearrange("(p j) -> p j", p=P)
    sts_r = sts_out.rearrange("(p j) -> p j", p=P)

    ccols = L // IN_CHUNKS
    jg = J // OUT_GROUPS

    from contextlib import ExitStack

    with ExitStack() as ctx:
        X = ctx.enter_context(nc.sbuf_tensor([P, J * L], f32))
        trampm = ctx.enter_context(nc.sbuf_tensor([P, L], f32))
        r = ctx.enter_context(nc.sbuf_tensor([P, J], f32))
        e = ctx.enter_context(nc.sbuf_tensor([P, J], f32))
        fmin = ctx.enter_context(nc.sbuf_tensor([P, J], f32))
        sumA = ctx.enter_context(nc.sbuf_tensor([P, J], f32))
        ssum = ctx.enter_context(nc.sbuf_tensor([P, J], f32))
        first_f = ctx.enter_context(nc.sbuf_tensor([P, J], f32))
        sts = ctx.enter_context(nc.sbuf_tensor([P, J], f32))
        mlt = ctx.enter_context(nc.sbuf_tensor([P, J], f32))
        hard_f = ctx.enter_context(nc.sbuf_tensor([P, J], f32))
        hard_i = ctx.enter_context(nc.sbuf_tensor([P, J], i32))
        s_tr = ctx.enter_context(nc.semaphore(name="s_tr"))
        s_chunk = [
            ctx.enter_context(nc.semaphore(name=f"s_chunk{c}"))
            for c in range(IN_CHUNKS)
        ]
        s_sp = ctx.enter_context(nc.semaphore(name="s_sp"))
        s_dec = ctx.enter_context(nc.semaphore(name="s_dec"))
        s_out = ctx.enter_context(nc.semaphore(name="s_out"))
        block = ctx.enter_context(nc.Block())
        X3 = X[:].rearrange("p (j t) -> p j t", t=L)
        X2 = X[:]

        @block.sync
        def _(sync):
            sync.dma_start(out=trampm[:], in_=trampm_in[:, :]).then_inc(s_tr, 16)
            for c in range(IN_CHUNKS):
                c0, c1 = c * ccols, (c + 1) * ccols
                sync.dma_start(
                    out=X3[:, :, c0:c1], in_=I_r[:, :, c0:c1]
                ).then_inc(s_chunk[c], 16)
            # spike groups out as they convert
            for g in range(OUT_GROUPS):
                sync.wait_ge(s_sp, g + 1)
                sync.dma_start(
                    out=S_r[:, g * jg : (g + 1) * jg, :],
                    in_=X3[:, g * jg : (g + 1) * jg, :],
                ).then_inc(s_out, 16)
            sync.wait_ge(s_dec, 1)
            sync.dma_start(out=hard_r[:, :], in_=hard_i[:]).then_inc(s_out, 16)
            sync.wait_ge(s_dec, 2)
            sync.dma_start(out=ssum_r[:, :], in_=ssum[:]).then_inc(s_out, 16)
            sync.dma_start(out=sts_r[:, :], in_=sts[:]).then_inc(s_out, 16)

        @block.vector
        def _(vector):
            vector.wait_ge(s_chunk[0], 16)
            # r_0 = u_0 * (u_0 < 1); u_0 = I_0 already in place
            nc.vector.scalar_tensor_tensor(
                out=r[:], in0=X3[:, :, 0], scalar=TH, in1=X3[:, :, 0],
                op0=Alu.is_lt, op1=Alu.mult,
            )
            for t in range(1, L):
                if t % ccols == 0:
                    vector.wait_ge(s_chunk[t // ccols], 16)
                ut = X3[:, :, t]
                nc.vector.scalar_tensor_tensor(
                    out=e[:], in0=r[:], scalar=NEG_INV_TAU, in1=ut,
                    op0=Alu.mult, op1=Alu.add,
                )
                nc.vector.tensor_tensor(out=ut, in0=r[:], in1=e[:], op=Alu.add)
                nc.vector.scalar_tensor_tensor(
                    out=r[:], in0=ut, scalar=TH, in1=ut,
                    op0=Alu.is_lt, op1=Alu.mult,
                )

            # X <- (u >= 1) * (t - 400), per row-block, in place
            vector.wait_ge(s_tr, 16)
            for j in range(J):
                blk = X2[:, j * L : (j + 1) * L]
                nc.vector.scalar_tensor_tensor(
                    out=blk, in0=blk, scalar=TH, in1=trampm[:],
                    op0=Alu.is_ge, op1=Alu.mult,
                )
            nc.vector.tensor_reduce(
                out=fmin[:], in_=X3, axis=Ax.X, op=Alu.min
            )
            nc.vector.tensor_reduce(
                out=sumA[:], in_=X3, axis=Ax.X, op=Alu.add
            )
            # spikes in place, grouped: s = (X < -0.5)
            for g in range(OUT_GROUPS):
                nc.vector.tensor_scalar(
                    out=X3[:, g * jg : (g + 1) * jg, :],
                    in0=X3[:, g * jg : (g + 1) * jg, :],
                    scalar1=-0.5, scalar2=None, op0=Alu.is_lt,
                ).then_inc(s_sp, 1)
            nc.vector.tensor_reduce(
                out=ssum[:], in_=X3, axis=Ax.X, op=Alu.add
            )

            # decode: first = fmin + 400; hard = first * (first < 400) as i32
            nc.vector.tensor_scalar(
                out=first_f[:], in0=fmin[:], scalar1=float(L), scalar2=None,
                op0=Alu.add,
            )
            nc.vector.tensor_scalar(
                out=mlt[:], in0=first_f[:], scalar1=float(L) - 0.5,
                scalar2=None, op0=Alu.is_lt,
            )
            nc.vector.tensor_tensor(
                out=hard_f[:], in0=first_f[:], in1=mlt[:], op=Alu.mult
            )
            nc.vector.tensor_copy(out=hard_i[:], in_=hard_f[:]).then_inc(
                s_dec, 1
            )
            # sum(s*t) = sumA + 400*sum(s); the final soft division is a
            # [B]-sized host op on these exact integer-valued sums.
            nc.vector.scalar_tensor_tensor(
                out=sts[:], in0=ssum[:], scalar=float(L), in1=sumA[:],
                op0=Alu.mult, op1=Alu.add,
            ).then_inc(s_dec, 1)

    return nc


def _get_program():
    if "nc" not in _PROGRAM_CACHE:
        _PROGRAM_CACHE["nc"] = _build_program()
    return _PROGRAM_CACHE["nc"]


def kernel(**inputs):
    I = np.ascontiguousarray(np.asarray(inputs["I"], dtype=np.float32))
    assert I.shape == (B, L), I.shape

    nc = _get_program()

    trampm = np.ascontiguousarray(
        np.tile(np.arange(L, dtype=np.float32) - np.float32(L), (P, 1))
    )
    in_maps = [
        {"I": I[c * BS : (c + 1) * BS], "trampm": trampm} for c in range(NCORES)
    ]

    from concourse.bass_utils import run_bass_kernel_spmd

    res = run_bass_kernel_spmd(nc, in_maps, core_ids=list(range(NCORES)))
    results = res.results

    spikes = np.concatenate([r["spikes"] for r in results], axis=0)
    hard = np.concatenate([r["hard"] for r in results], axis=0).astype(np.int32)
    ssum = np.concatenate([r["ssum"] for r in results], axis=0)
    sts = np.concatenate([r["sts"] for r in results], axis=0)
    f32 = np.float32
    soft = (sts / (ssum + f32(1e-6))).astype(f32)
    return (spikes, hard, soft)
